# revision 43
# baseline (speedup 1.0000x reference)
"""Trainium2 Bass kernel v3 for nn_MultiHeadAttention_47631187313085.

Math (reference):
    Q[h] = (XQ @ WQ_comb) @ WQh[h]          # folded: XQ @ (WQ_comb @ WQh[h])
    scores[h] = Q[h] @ K[h].T / sqrt(dk)    # [q, s]
    attn = softmax(scores, axis=q)          # normalize over the QUERY axis
    heads[h] = attn[h] @ V[h]               # [q, dk]
    out = concat(heads) @ WO

Because softmax normalizes over q (not the contracted axis s), the
normalizer D[s] = sum_q exp(S[q,s] - c[s]) can be folded into V:
    out = EXP(S - c) @ (V * 1/D)   for ANY per-key offset c[s].
c[s] only needs to be within ~±45 of the true column max for bf16
representability.  The host picks 64 candidate queries per head from a
cheap proxy (a = XQ @ (wq2 @ (X̄K @ wk2))) plus a strided sample,
projects them, and the device computes c[s] = max over candidates.

v3 changes vs v2 (541us -> ~400us fast-mode equivalent):
  - group=2 AV accumulation with a pending-queue drain: the E-tile
    working set fits the 12-buffer pool, removing the ~6-8us ACT
    stalls at every group boundary that v2's group=4 structure had.
  - The first TWO groups' exps interleave with the projection head,
    and K-projection beyond the first 1024 columns (plus its
    candidate-max offsets) is deferred into the steady state, two
    groups ahead of use — the head no longer gates on 16MB of DMA.
  - ACT's DMA queue carries no transfers during the stream (pure exp);
    x loads ride the SP + gpsimd(SWDGE) queues; weights are
    pre-transposed on the host so every weight DMA is contiguous.
    The exp ACT table is warmed at kernel start.
  - Tail: the final AV accumulation renders straight to an fp8e4m3
    staging tile; ONE AllGather (fp8, 2MB out) + column-sharded WO
    whose ccr reads (fp8, mixed fp8xfp16 matmul) split across two DMA
    queues; outT in fp16.  (An AllToAll query-resharding variant,
    tail_mode="ata", measured SLOWER on this runtime — the collective
    constant dominates - and is kept only for reference.)

Sharding: tensor-parallel over heads, 2 heads per core; AllGather of
fp8 head outputs; each core computes out[:, 128c:128(c+1)].
"""

import sys

sys.path.insert(0, "/opt/trn_rl_repo")

import numpy as np
import ml_dtypes

FP16 = np.float16
BF16 = ml_dtypes.bfloat16

H = 16
D_MODEL = 1024
D_K = 64
SEQ = 4096
N_CORES = 8
HPC = H // N_CORES  # heads per core
K2 = HPC * D_K      # 128: per-core concat width
NCAND = 64          # candidate queries per head for the offset c[s]


TAIL_MODE = "ag"  # "ag": AllGather + column-sharded WO; "ata": AllToAll


def build_program(D, S, n_devices, group=2, fake_ag=False, reps=1,
                  av_start=True, ts_accum=True, cand_c=True,
                  use_bf16=True, dve_mul=True, upto=99, act_accum=False,
                  xtile_bufs=2, head_groups=2, drain_max=6,
                  tail_mode=None, ag_split=1, stage8=True):
    tail_mode = tail_mode or TAIL_MODE
    """Build the SPMD Bass program (identical on all cores; data differs).

    Per-core external inputs (fp16):
      xqt/xkt/xvt : [D, S]        transposed activations (replicated)
      wq2/wk2/wv2 : [128, EC*K2]  folded per-core weights, p-major layout
                                  (wq2 also carries the 1/sqrt(dk) scale)
      qc2         : [K2, NCAND]   projected candidate queries (per head)
      wo_c        : [128, NKB*OCB*128]  FULL WO, p-major [p, kb, ocb, col]
    Output:
      outT : [CC, QW] f32   (out[qw_c, :].T for this core's query window)
    """
    import concourse.bacc as bacc
    import concourse.mybir as mybir
    import concourse.tile as tile

    f32 = mybir.dt.float32
    fp16 = mybir.dt.float16
    fp8 = mybir.dt.float8e4
    bf16 = mybir.dt.bfloat16 if use_bf16 else mybir.dt.float16
    EXP = mybir.ActivationFunctionType.Exp

    EC = D // 128           # contraction chunks for the projections
    SC = S // 128           # key/seq chunks
    QB = S // 512           # query blocks of 512
    SH = 1024               # exp tile width (2 psum banks)
    NSH = S // SH           # exp tiles per (sc, h)
    NG = SC // group        # AV accumulation groups
    NPAIR = group * HPC     # (sc, h) pairs per group
    CC = n_devices * K2     # concat width (= D for the real problem)
    NKB = CC // 128         # contraction blocks for WO
    OCB = CC // 128         # output column blocks for WO
    QW = S // n_devices     # per-core query window (512)

    nc = bacc.Bacc("TRN2", target_bir_lowering=False, num_devices=n_devices,
                   enable_partition_id=False)

    xqt = nc.dram_tensor("xqt", [D, S], fp16, kind="ExternalInput")
    xkt = nc.dram_tensor("xkt", [D, S], fp16, kind="ExternalInput")
    xvt = nc.dram_tensor("xvt", [D, S], fp16, kind="ExternalInput")
    wq2 = nc.dram_tensor("wq2", [128, EC * K2], fp16, kind="ExternalInput")
    wk2 = nc.dram_tensor("wk2", [128, EC * K2], fp16, kind="ExternalInput")
    wv2 = nc.dram_tensor("wv2", [128, EC * K2], fp16, kind="ExternalInput")
    qc2 = nc.dram_tensor("qc2", [K2, NCAND], fp16, kind="ExternalInput")
    if tail_mode == "ata":
        wo_c = nc.dram_tensor("wo_c", [128, NKB * OCB * 128], fp16,
                              kind="ExternalInput")
        outT = nc.dram_tensor("outT", [CC, QW], f32, kind="ExternalOutput")
    else:
        wo_c = nc.dram_tensor("wo_c", [CC, 128], fp16, kind="ExternalInput")
        outT = nc.dram_tensor("outT", [128, S], fp16, kind="ExternalOutput")

    with tile.TileContext(nc) as tc:
        with (
            tc.tile_pool(name="const", bufs=1) as const,
            tc.tile_pool(name="main", bufs=1) as main,
            tc.tile_pool(name="xs", bufs=2) as xs,
            tc.tile_pool(name="ep", bufs=3 * group * HPC) as ep,
            tc.tile_pool(name="vp", bufs=2) as vpp,
            tc.tile_pool(name="sm", bufs=20 if act_accum else 8) as sm,
            tc.tile_pool(name="outp", bufs=2) as outp,
            tc.tile_pool(name="dram", bufs=1, space="DRAM") as dram,
        ):
            # ---- weights to SBUF (gpsimd SWDGE queue: keeps SP/DVE/ACT
            # queues free for the x loads and the exp stream) ----
            wq2_sb = const.tile([128, EC, K2], fp16)
            wk2_sb = const.tile([128, EC, K2], fp16)
            wv2_sb = const.tile([128, EC, K2], fp16)
            qc2_sb = const.tile([128, NCAND], fp16)
            if tail_mode == "ata":
                wo_sb = const.tile([128, NKB, OCB, 128], fp16)
            else:
                wo_sb = const.tile([128, NKB, 128], fp16)
            nc.gpsimd.dma_start(wq2_sb[:], wq2.rearrange("p (o k) -> p o k", o=EC))
            nc.gpsimd.dma_start(wk2_sb[:], wk2.rearrange("p (o k) -> p o k", o=EC))
            nc.gpsimd.dma_start(qc2_sb[:], qc2[:, :])
            nc.gpsimd.dma_start(wv2_sb[:], wv2.rearrange("p (o k) -> p o k", o=EC))
            # wo_sb's load is emitted mid-stream (steady state, queue idle)
            # so it doesn't delay the early xtile loads on this queue
            wo_loaded = [False]

            def emit_wo_load():
                if wo_loaded[0]:
                    return
                wo_loaded[0] = True
                if tail_mode == "ata":
                    nc.gpsimd.dma_start(
                        wo_sb[:],
                        wo_c.rearrange("p (a b k) -> p a b k", a=NKB, b=OCB))
                else:
                    nc.gpsimd.dma_start(
                        wo_sb[:], wo_c.rearrange("(o p) k -> p o k", p=128))

            sps = tc.alloc_tile_pool(name="sps", bufs=3, space="PSUM")
            avs = tc.alloc_tile_pool(name="avs", bufs=2, space="PSUM")
            # warm the ACT exp table (its ~2.7us load would otherwise sit
            # on the first real exp's critical path)
            warm = sm.tile([128, 1], f32, tag="rden", name="warm")
            nc.vector.memset(warm[:], 0.0)
            nc.scalar.activation(warm[:], warm[:],
                                 mybir.ActivationFunctionType.Exp)
            for _rep in range(reps):
                # ---- projections (+ per-chunk candidate offsets) ----
                # q2t and heads2 share a 2-buffer tag with rep-alternating
                # allocation order: rep r+1's q2t lands in rep r's heads2
                # buffer (free right after the final flush), so the next
                # rep's Q projection overlaps this rep's AllGather/WO tail.
                def qh_tile(name):
                    return main.tile([128, S], fp16, tag="qh", name=name,
                                     bufs=2)

                if _rep % 2 == 0:
                    q2t = qh_tile("q2t")
                    heads2 = qh_tile("heads2")
                else:
                    heads2 = qh_tile("heads2")
                    q2t = qh_tile("q2t")
                k2t = main.tile([128, S], fp16)
                v2 = main.tile([128, SC, K2], fp16)
                cbias = main.tile([128, SC, HPC], f32)  # -c[s] per (sc, h)
                xq3 = xqt.rearrange("(o p) q -> p o q", p=128)
                xk3 = xkt.rearrange("(o p) q -> p o q", p=128)
                xv3 = xvt.rearrange("(o p) s -> p o s", p=128)
                # fp8 staging target for the final AV accumulation (ag tail)
                cc8 = (main.tile([128, S], fp8, name="cc8")
                       if tail_mode == "ag" and stage8 and upto >= 4 else None)

                def emit_proj_q(qb):
                    # 1024-wide xtiles (2KB contiguous runs per partition —
                    # the DMA-efficiency threshold); DMA on even qb covers
                    # the odd qb too.  q rides the SP HWDGE queue, k rides
                    # the gpsimd queue: ACT's queue carries no transfers.
                    sub = qb % 2
                    if sub == 0:
                        proj_xt["q"] = xs.tile([128, EC, 1024], fp16,
                                               tag="xqk", name="xtile",
                                               bufs=xtile_bufs)
                        nc.sync.dma_start(
                            proj_xt["q"][:],
                            xq3[:, :, (qb // 2) * 1024:(qb // 2 + 1) * 1024])
                    ps = avs.tile([128, 512], f32, tag="av", name="ps_qk")
                    for e in range(EC):
                        nc.tensor.matmul(
                            ps[:], wq2_sb[:, e, :],
                            proj_xt["q"][:, e, sub * 512:(sub + 1) * 512],
                            start=(e == 0), stop=(e == EC - 1),
                        )
                    nc.vector.tensor_copy(q2t[:, qb * 512:(qb + 1) * 512],
                                          ps[:])

                def emit_proj_k(kqb):
                    # K projection + candidate offsets for 512 key cols;
                    # only the first 1024 cols happen in the head — the rest
                    # stream during the attention phase, two groups ahead
                    # of their first use.
                    xkt_t = xs.tile([128, EC, 512], fp16, tag="xk",
                                    name="xktile", bufs=2)
                    nc.gpsimd.dma_start(
                        xkt_t[:], xk3[:, :, kqb * 512:(kqb + 1) * 512])
                    ps = avs.tile([128, 512], f32, tag="av", name="ps_qk")
                    for e in range(EC):
                        nc.tensor.matmul(
                            ps[:], wk2_sb[:, e, :], xkt_t[:, e, :],
                            start=(e == 0), stop=(e == EC - 1),
                        )
                    nc.vector.tensor_copy(k2t[:, kqb * 512:(kqb + 1) * 512],
                                          ps[:])
                    # candidate scores: c[s] = max over candidates
                    for sc in range(kqb * SC // QB, (kqb + 1) * SC // QB) if upto >= 2 else ():
                        for h in range(HPC):
                            ps = avs.tile([128, 512], f32, tag="av",
                                          name="ps_c")
                            nc.tensor.matmul(
                                ps[:, :NCAND],
                                k2t[h * 64:(h + 1) * 64,
                                    sc * 128:(sc + 1) * 128],
                                qc2_sb[h * 64:(h + 1) * 64, :],
                                start=True, stop=True,
                            )
                            nc.vector.tensor_reduce(
                                cbias[:, sc, h:h + 1],
                                ps[:, :NCAND],
                                axis=mybir.AxisListType.X,
                                op=mybir.AluOpType.max, negate=True,
                            )

                def emit_score_exp2(g, scl, t, ets, accs=(None, None)):
                    # h0/h1 matmuls interleaved 1:1 so consecutive PE
                    # instructions target different row groups (64-row
                    # contraction halves) and run concurrently on silicon
                    sc = g * group + scl
                    sp2 = [sps.tile([128, SH], f32, tag="spsum", name="sp")
                           for _ in range(HPC)]
                    for m in range(SH // 512):
                        qo = t * SH + m * 512
                        for h in range(HPC):
                            nc.tensor.matmul(
                                sp2[h][:, m * 512:(m + 1) * 512],
                                k2t[h * 64:(h + 1) * 64,
                                    sc * 128:(sc + 1) * 128],
                                q2t[h * 64:(h + 1) * 64, qo:qo + 512],
                                start=True, stop=True,
                            )
                    for h in range(HPC):
                        nc.scalar.activation(
                            ets[h][:, t * SH:(t + 1) * SH], sp2[h][:], EXP,
                            bias=cbias[:, sc, h:h + 1],
                            accum_out=accs[h],
                        )

                def emit_den(pair, et, den, accq=None):
                    # D'[s] for this pair = sum_q E.
                    if act_accum:
                        nc.vector.tensor_add(
                            den[:, pair:pair + 1], accq[:, 0, :], accq[:, 1, :])
                        for t in range(2, NSH):
                            nc.vector.tensor_add(
                                den[:, pair:pair + 1],
                                den[:, pair:pair + 1], accq[:, t, :])
                    elif ts_accum:
                        nc.vector.tensor_scalar(
                            et[:, :], et[:, :], 1.0, 0.0, mybir.AluOpType.mult,
                            mybir.AluOpType.add,
                            accum_out=den[:, pair:pair + 1],
                        )
                    else:
                        nc.vector.tensor_reduce(
                            den[:, pair:pair + 1], et[:, :],
                            axis=mybir.AxisListType.X, op=mybir.AluOpType.add,
                        )

                def emit_sc_pair(g, scl, den):
                    ets = [ep.tile([128, S], bf16, tag="E", name="et")
                           for _ in range(HPC)]
                    accqs = [None] * HPC
                    if act_accum:
                        accqs = [sm.tile([128, NSH, 1], f32, tag="accq",
                                         name="accq") for _ in range(HPC)]
                    for t in range(NSH):
                        emit_score_exp2(
                            g, scl, t, ets,
                            tuple(a[:, t, :] if act_accum else None
                                  for a in accqs))
                    for h in range(HPC):
                        emit_den(scl * HPC + h, ets[h], den, accqs[h])
                    return ets

                def emit_group_tail(g, e_tiles, den):
                    rden = sm.tile([128, NPAIR], f32, tag="rden", name="rden")
                    nc.vector.reciprocal(rden[:], den[:])
                    vpt = vpp.tile([128, group, K2], bf16, tag="vp",
                                   name="vpt")
                    for scl in range(group):
                        for h in range(HPC):
                            if dve_mul:
                                nc.vector.tensor_scalar_mul(
                                    vpt[:, scl, h * 64:(h + 1) * 64],
                                    v2[:, g * group + scl, h * 64:(h + 1) * 64],
                                    rden[:, scl * HPC + h:scl * HPC + h + 1],
                                )
                            else:
                                nc.scalar.mul(
                                    vpt[:, scl, h * 64:(h + 1) * 64],
                                    v2[:, g * group + scl, h * 64:(h + 1) * 64],
                                    rden[:, scl * HPC + h:scl * HPC + h + 1],
                                )
                    return vpt

                def emit_av_qb(g, e_tiles, vpt, qb):
                    # both heads packed per psum bank via column tiling
                    av = avs.tile([128, 512], f32, tag="av", name="av")
                    if not av_start:
                        nc.vector.memset(av[:], 0.0)
                    for scl in range(group):
                        for h in range(HPC):
                            nc.tensor.matmul(
                                av[h * 64:(h + 1) * 64, :],
                                vpt[:, scl, h * 64:(h + 1) * 64],
                                e_tiles[(scl, h)][:, qb * 512:(qb + 1) * 512],
                                start=(av_start and scl == 0),
                                stop=(scl == group - 1 and h == HPC - 1),
                                skip_group_check=True,
                                tile_position=(0, h * 64),
                            )
                    src = heads2[:, qb * 512:(qb + 1) * 512]
                    if g == NG - 1 and cc8 is not None:
                        # final accumulation renders straight to the fp8
                        # staging tile (halves AllGather + WO read bytes)
                        nc.vector.tensor_add(
                            cc8[:, qb * 512:(qb + 1) * 512], src, av[:])
                    elif g == 0:
                        nc.vector.tensor_copy(src, av[:])
                    else:
                        nc.vector.tensor_add(src, src, av[:])

                def emit_xvg_dma(g):
                    # one group-wide xv DMA (s-chunks, 1KB runs) on the SP
                    # queue, streamed during the attention phase
                    xvg = xs.tile([128, EC, group * 128], fp16, tag="xv",
                                  name="xvg", bufs=1)
                    nc.sync.dma_start(
                        xvg[:], xv3[:, :, g * group * 128:(g + 1) * group * 128])
                    return xvg

                def emit_vproj_chunk(xvg, g, scl):
                    # deferred V projection, one s-chunk at a time so the PE
                    # work spreads across the group instead of bunching at
                    # the boundary (which starved ACT of score tiles)
                    sc = g * group + scl
                    ps = avs.tile([128, 512], f32, tag="av", name="ps_v")
                    for e in range(EC):
                        nc.tensor.matmul(
                            ps[:, :K2],
                            xvg[:, e, scl * 128:(scl + 1) * 128],
                            wv2_sb[:, e, :],
                            start=(e == 0), stop=(e == EC - 1),
                        )
                    nc.vector.tensor_copy(v2[:, sc, :], ps[:, :K2])

                # ---- AV work queue: groups whose den/vpt are ready get
                # their 8 AV q-blocks drained a few at a time per pair, so
                # PE work spreads evenly and E tiles retire steadily ----
                pending = []  # [g, tiles, vpt, next_qb]

                def drain_av(n):
                    done = 0
                    while pending and done < n:
                        ent = pending[0]
                        emit_av_qb(ent[0], ent[1], ent[2], qb=ent[3])
                        ent[3] += 1
                        done += 1
                        if ent[3] == QB:
                            pending.pop(0)

                # ---- projections, interleaved with the first head_groups
                # groups' attention.  Their exp t-rounds are emitted as soon
                # as the q2t columns they read are projected, so the ACT exp
                # stream saturates while the (DMA-bound) projection phase is
                # still streaming inputs.
                HG = min(head_groups, NG) if upto >= 3 else 0
                # k columns needed by steady group g are [g*group*128,
                # (g+1)*group*128); head covers kqb 0..HKQB-1, the rest are
                # emitted two groups ahead of first use (group==2 only —
                # other group sizes project everything in the head).
                if upto >= 3 and group == 2:
                    HKQB = 2
                else:
                    HKQB = QB
                ksched = {}  # steady group -> kqb to project during it
                for kqb in range(HKQB, QB):
                    ksched[(kqb * 512) // (group * 128) - 2] = kqb
                proj_xt = {}
                hg_tiles = {}  # (g, scl, h) -> E tile
                hg_accq = {}
                hg_dens = {}
                xvgs = {}
                for qb in range(QB):
                    emit_proj_q(qb)
                    if qb < HKQB:
                        emit_proj_k(qb)
                    if upto < 3:
                        continue
                    if qb == 0:
                        for g in range(HG):
                            xvgs[g] = emit_xvg_dma(g)
                            hg_dens[g] = sm.tile([128, NPAIR], f32,
                                                 tag="den", name="den")
                    if qb % 2 == 1:
                        t = qb // 2
                        for g in range(HG):
                            for scl in range(group):
                                for h in range(HPC):
                                    if (g, scl, h) not in hg_tiles:
                                        hg_tiles[(g, scl, h)] = ep.tile(
                                            [128, S], bf16, tag="E",
                                            name="et")
                                        if act_accum:
                                            hg_accq[(g, scl, h)] = sm.tile(
                                                [128, NSH, 1], f32,
                                                tag="accq", name="accq")
                                emit_score_exp2(
                                    g, scl, t,
                                    [hg_tiles[(g, scl, h)]
                                     for h in range(HPC)],
                                    tuple(hg_accq[(g, scl, h)][:, t, :]
                                          if act_accum else None
                                          for h in range(HPC)))
                        i = qb // 2
                        if i < HG * group:
                            emit_vproj_chunk(xvgs[i // group], i // group,
                                             i % group)
                if upto < 3:
                    continue
                for g in range(HG):
                    for scl in range(group):
                        for h in range(HPC):
                            emit_den(scl * HPC + h, hg_tiles[(g, scl, h)],
                                     hg_dens[g], hg_accq.get((g, scl, h)))
                    tiles_g = {(scl, h): hg_tiles[(g, scl, h)]
                               for scl in range(group) for h in range(HPC)}
                    vpt_g = emit_group_tail(g, tiles_g, hg_dens[g])
                    pending.append([g, tiles_g, vpt_g, 0])

                for g in range(HG, NG):
                    e_tiles = {}
                    xvg = emit_xvg_dma(g)
                    if g in ksched:
                        emit_proj_k(ksched[g])
                    if g == HG:
                        emit_wo_load()
                    den = sm.tile([128, NPAIR], f32, tag="den", name="den")
                    for scl in range(group):
                        ets = emit_sc_pair(g, scl, den)
                        for h in range(HPC):
                            e_tiles[(scl, h)] = ets[h]
                        emit_vproj_chunk(xvg, g, scl)
                        drain_av(drain_max)
                    vpt = emit_group_tail(g, e_tiles, den)
                    pending.append([g, e_tiles, vpt, 0])

                # ---- final AV flush + cross-core redistribution ----
                if tail_mode == "ata":
                    # heads2[:, qw_d] goes to core d; one small (1MB)
                    # AllToAll; each core then computes the full 1024-wide
                    # WO for its own 512-query window.
                    ata_in = dram.tile([CC, QW], fp16, name="ata_in")
                    ata_out = dram.tile([CC, QW], fp16, name="ata_out")
                    NQW = QW // 512

                    def stage_qb(qb):
                        d, lo = qb // NQW, (qb % NQW) * 512
                        nc.sync.dma_start(
                            ata_in[d * 128:(d + 1) * 128, lo:lo + 512],
                            heads2[:, qb * 512:(qb + 1) * 512])
                else:
                    # v2-style: stage heads2 to DRAM, AllGather the full
                    # concat in ag_split row parts (WO matmuls over part 0
                    # overlap the later gathers; wo_c rows pre-permuted on
                    # the host to match), column-sharded WO.
                    cc_dt = fp8 if stage8 else fp16
                    cc_src = cc8 if stage8 else heads2
                    cc_in = dram.tile([128, S], cc_dt, name="cc_in")
                    space = "Local" if (fake_ag or n_devices <= 4) else "Shared"
                    RPP = 128 // ag_split
                    cc_part = [dram.tile([CC // ag_split, S], cc_dt,
                                         addr_space=space, name=f"cc_part{i}")
                               for i in range(ag_split)]

                    def stage_qb(qb):
                        nc.sync.dma_start(
                            cc_in[:, qb * 512:(qb + 1) * 512],
                            cc_src[:, qb * 512:(qb + 1) * 512])

                while pending:
                    last = len(pending) == 1
                    qb = pending[0][3]
                    drain_av(1)
                    if last and upto >= 4:
                        stage_qb(qb)
                if upto < 4:
                    continue

                if tail_mode == "ata":
                    if fake_ag:
                        nc.sync.dma_start(ata_out[:, :], ata_in[:, :])
                    else:
                        nc.gpsimd.collective_compute(
                            "AllToAll", mybir.AluOpType.bypass,
                            replica_groups=[list(range(n_devices))],
                            ins=[ata_in[:, :].opt()],
                            outs=[ata_out[:, :].opt()],
                        )
                else:
                    for i in range(ag_split):
                        part = cc_in[i * RPP:(i + 1) * RPP, :]
                        if fake_ag:
                            nc.sync.dma_start(cc_part[i][:RPP, :], part)
                        else:
                            nc.gpsimd.collective_compute(
                                "AllGather", mybir.AluOpType.bypass,
                                replica_groups=[list(range(n_devices))],
                                ins=[part.opt()], outs=[cc_part[i].opt()],
                            )

                if upto < 5:
                    continue
                if tail_mode == "ata":
                    # ---- WO: full 1024 columns for this core's 512-query
                    # window, contraction over the gathered concat.  att_sb
                    # borrows an E-pool buffer (E tiles are dead here) ----
                    att_sb = ep.tile([128, NKB, QW], fp16, tag="E", name="att")
                    ata3 = ata_out.rearrange("(o p) q -> p o q", p=128)
                    for kb in range(NKB):
                        nc.sync.dma_start(att_sb[:, kb, :], ata3[:, kb, :])
                    for j in range(OCB // 2):
                        pq = sps.tile([128, 1024], f32, tag="spsum", name="pq")
                        for kb in range(NKB):
                            for i in (0, 1):
                                nc.tensor.matmul(
                                    pq[:, i * 512:(i + 1) * 512],
                                    wo_sb[:, kb, 2 * j + i, :],
                                    att_sb[:, kb, :],
                                    start=(kb == 0), stop=(kb == NKB - 1),
                                )
                        for i in (0, 1):
                            ocb = 2 * j + i
                            osb = outp.tile([128, 512], f32, tag="osb",
                                            name="osb")
                            nc.vector.tensor_copy(
                                osb[:], pq[:, i * 512:(i + 1) * 512])
                            nc.sync.dma_start(
                                outT[ocb * 128:(ocb + 1) * 128, :], osb[:])
                else:
                    # ---- WO: this core's 128 output columns over all S
                    # queries; 2048-wide ccr chunks on the (now idle) ACT
                    # HWDGE queue; 4 psum chains in two 1024-wide tiles ----
                    QWW = 4  # 512-blocks per ccr chunk
                    WCC = QWW * 512
                    cpp = NKB // ag_split  # 128-row chunks per AG part
                    for qbo in range(QB // QWW):
                        pqs = [sps.tile([128, 1024], f32, tag="spsum",
                                        name="pq") for _ in range(QWW // 2)]
                        for kb in range(NKB):
                            ccr = xs.tile([128, WCC], cc_dt, tag="ccr",
                                          name="ccr", bufs=2)
                            qw = slice(qbo * WCC, (qbo + 1) * WCC)
                            eng = nc.scalar if kb % 2 == 0 else nc.gpsimd
                            eng.dma_start(
                                ccr[:], cc_part[kb // cpp][
                                    (kb % cpp) * 128:(kb % cpp) * 128 + 128,
                                    qw])
                            for qi in range(QWW):
                                nc.tensor.matmul(
                                    pqs[qi // 2][:, (qi % 2) * 512:(qi % 2 + 1) * 512],
                                    wo_sb[:, kb, :],
                                    ccr[:, qi * 512:(qi + 1) * 512],
                                    start=(kb == 0), stop=(kb == NKB - 1),
                                )
                        for qi in range(QWW):
                            qb = qbo * QWW + qi
                            osb = outp.tile([128, 512], fp16, tag="osb",
                                            name="osb")
                            nc.vector.tensor_copy(
                                osb[:], pqs[qi // 2][:, (qi % 2) * 512:(qi % 2 + 1) * 512])
                            nc.sync.dma_start(
                                outT[:, qb * 512:(qb + 1) * 512], osb[:])
            avs.release()
            sps.release()

    nc.compile()
    return nc


def make_core_inputs(XQ, XK, XV, WQ_comb, WK_comb, WV_comb, WQh, WKh, WVh, WO,
                     n_cores=N_CORES, hpc=HPC, tail_mode=None, ag_split=1):
    """Host-side shard/layout prep. Returns in_maps for run_bass_kernel_spmd."""
    tail_mode = tail_mode or TAIL_MODE
    f32 = np.float32
    xqt = np.ascontiguousarray(np.asarray(XQ, f32).T).astype(FP16)
    xkt = np.ascontiguousarray(np.asarray(XK, f32).T).astype(FP16)
    xvt = np.ascontiguousarray(np.asarray(XV, f32).T).astype(FP16)
    XQ = np.asarray(XQ, f32)
    xkbar = np.asarray(XK, f32).mean(axis=0)
    WQ_comb = np.asarray(WQ_comb, f32)
    WK_comb = np.asarray(WK_comb, f32)
    WV_comb = np.asarray(WV_comb, f32)
    WQh, WKh, WVh = np.asarray(WQh, f32), np.asarray(WKh, f32), np.asarray(WVh, f32)
    WO = np.asarray(WO, f32)
    strided = np.arange(0, XQ.shape[0], 128)
    EC = D_MODEL // 128
    NKB = (n_cores * K2) // 128

    def pmajor(w):  # [D, K2] -> [128, EC*K2] with p-major contiguous layout
        return np.ascontiguousarray(
            w.reshape(EC, 128, K2).transpose(1, 0, 2).reshape(128, EC * K2))

    if tail_mode == "ata":
        # full WO, p-major [p, kb, ocb, col], replicated to every core
        wo_full = np.ascontiguousarray(
            WO.reshape(NKB, 128, NKB, 128).transpose(1, 0, 2, 3).reshape(128, -1)
        ).astype(FP16)
    else:
        # WO row permutation matching the N-way split AllGather layout:
        # [all cores' local rows of part 0; ...; part N-1]
        rpp = 128 // ag_split
        wo_perm = np.concatenate(
            [np.arange(c * 128 + p * rpp, c * 128 + (p + 1) * rpp)
             for p in range(ag_split) for c in range(n_cores)])

    in_maps = []
    for c in range(n_cores):
        hs = slice(c * hpc, (c + 1) * hpc)
        # stack this core's heads along columns, then fold the combined proj;
        # the softmax 1/sqrt(dk) goes into the Q weights
        wq2 = (WQ_comb @ np.concatenate(list(WQh[hs]), axis=1)) / np.sqrt(D_K)
        wk2 = WK_comb @ np.concatenate(list(WKh[hs]), axis=1)
        wv2 = WV_comb @ np.concatenate(list(WVh[hs]), axis=1)
        k2 = wq2.shape[1]
        wq2_16, wk2_16 = wq2.astype(FP16), wk2.astype(FP16)
        # candidate queries per head: both tails of the cheap proxy
        # a[q] = XQ @ (wq2_h @ (xkbar @ wk2_h)) plus a strided sample;
        # project them exactly like the device does (fp16 operands,
        # f32 accumulate, fp16 result)
        qc2 = np.empty((k2, NCAND), FP16)
        for h in range(hpc):
            cs = slice(h * D_K, (h + 1) * D_K)
            u = xkbar @ wk2[:, cs]
            a = XQ @ (wq2[:, cs] @ u)
            o = np.argsort(a)
            cand = np.concatenate([o[:16], o[-16:], strided])[:NCAND]
            qc = (XQ[cand].astype(FP16).astype(f32)
                  @ wq2_16[:, cs].astype(f32)).astype(FP16)  # [NCAND, 64]
            qc2[cs, :] = qc.T
        if tail_mode == "ata":
            wo_c = wo_full
        else:
            wo_c = np.ascontiguousarray(
                WO[wo_perm][:, c * k2:(c + 1) * k2]).astype(FP16)
        in_maps.append({
            "xqt": xqt, "xkt": xkt, "xvt": xvt,
            "wq2": pmajor(wq2).astype(FP16),
            "wk2": pmajor(wk2).astype(FP16),
            "wv2": pmajor(wv2).astype(FP16),
            "qc2": qc2,
            "wo_c": wo_c,
        })
    return in_maps


_PROGRAM_CACHE = {}


def _get_program(D, S, n_devices):
    key = (D, S, n_devices)
    if key not in _PROGRAM_CACHE:
        _PROGRAM_CACHE[key] = build_program(D, S, n_devices)
    return _PROGRAM_CACHE[key]


def kernel(XQ, XK, XV, WQ_comb, WK_comb, WV_comb, WQh, WKh, WVh, WO,
           _trace=False):
    from concourse.bass_utils import run_bass_kernel_spmd

    in_maps = make_core_inputs(XQ, XK, XV, WQ_comb, WK_comb, WV_comb,
                               WQh, WKh, WVh, WO)
    nc = _get_program(D_MODEL, SEQ, N_CORES)
    res = run_bass_kernel_spmd(nc, in_maps, core_ids=list(range(N_CORES)),
                               trace=_trace)
    out = np.empty((SEQ, D_MODEL), np.float32)
    if TAIL_MODE == "ata":
        QW = SEQ // N_CORES
        for c in range(N_CORES):
            out[c * QW:(c + 1) * QW, :] = res.results[c]["outT"].T
    else:
        for c in range(N_CORES):
            out[:, c * 128:(c + 1) * 128] = res.results[c]["outT"].T
    if _trace:
        kernel.last_results = res
    return out


# revision 44
# speedup vs baseline: 2.0378x; 2.0378x over previous
"""Trainium2 Bass kernel v3 for nn_MultiHeadAttention_47631187313085.

Math (reference):
    Q[h] = (XQ @ WQ_comb) @ WQh[h]          # folded: XQ @ (WQ_comb @ WQh[h])
    scores[h] = Q[h] @ K[h].T / sqrt(dk)    # [q, s]
    attn = softmax(scores, axis=q)          # normalize over the QUERY axis
    heads[h] = attn[h] @ V[h]               # [q, dk]
    out = concat(heads) @ WO

Because softmax normalizes over q (not the contracted axis s), the
normalizer D[s] = sum_q exp(S[q,s] - c[s]) can be folded into V:
    out = EXP(S - c) @ (V * 1/D)   for ANY per-key offset c[s].
c[s] only needs to be within ~±45 of the true column max for bf16
representability.  The host picks 64 candidate queries per head from a
cheap proxy (a = XQ @ (wq2 @ (X̄K @ wk2))) plus a strided sample,
projects them, and the device computes c[s] = max over candidates.

v3 changes vs v2 (541us -> ~400us fast-mode equivalent):
  - group=2 AV accumulation with a pending-queue drain: the E-tile
    working set fits the 12-buffer pool, removing the ~6-8us ACT
    stalls at every group boundary that v2's group=4 structure had.
  - The first TWO groups' exps interleave with the projection head,
    and K-projection beyond the first 1024 columns (plus its
    candidate-max offsets) is deferred into the steady state, two
    groups ahead of use — the head no longer gates on 16MB of DMA.
  - ACT's DMA queue carries no transfers during the stream (pure exp);
    x loads ride the SP + gpsimd(SWDGE) queues; weights are
    pre-transposed on the host so every weight DMA is contiguous.
    The exp ACT table is warmed at kernel start.
  - Tail: the final AV accumulation renders straight to an fp8e4m3
    staging tile; ONE AllGather (fp8, 2MB out) + column-sharded WO
    whose ccr reads (fp8, mixed fp8xfp16 matmul) split across two DMA
    queues; outT in fp16.  (An AllToAll query-resharding variant,
    tail_mode="ata", measured SLOWER on this runtime — the collective
    constant dominates - and is kept only for reference.)

Sharding: tensor-parallel over heads, 2 heads per core; AllGather of
fp8 head outputs; each core computes out[:, 128c:128(c+1)].
"""

import sys

sys.path.insert(0, "/opt/trn_rl_repo")

import numpy as np
import ml_dtypes

FP16 = np.float16
BF16 = ml_dtypes.bfloat16

H = 16
D_MODEL = 1024
D_K = 64
SEQ = 4096
N_CORES = 8
HPC = H // N_CORES  # heads per core
K2 = HPC * D_K      # 128: per-core concat width
NCAND = 64          # candidate queries per head for the offset c[s]


TAIL_MODE = "ag"  # "ag": AllGather + column-sharded WO; "ata": AllToAll


def build_program(D, S, n_devices, group=2, fake_ag=False, reps=1,
                  av_start=True, ts_accum=True, cand_c=True,
                  use_bf16=True, dve_mul=True, upto=99, act_accum=False,
                  xtile_bufs=2, head_groups=2, drain_max=4,
                  tail_mode=None, ag_split=1, stage8=True):
    tail_mode = tail_mode or TAIL_MODE
    """Build the SPMD Bass program (identical on all cores; data differs).

    Per-core external inputs (fp16):
      xqt/xkt/xvt : [D, S]        transposed activations (replicated)
      wq2/wk2/wv2 : [128, EC*K2]  folded per-core weights, p-major layout
                                  (wq2 also carries the 1/sqrt(dk) scale)
      qc2         : [K2, NCAND]   projected candidate queries (per head)
      wo_c        : [128, NKB*OCB*128]  FULL WO, p-major [p, kb, ocb, col]
    Output:
      outT : [CC, QW] f32   (out[qw_c, :].T for this core's query window)
    """
    import concourse.bacc as bacc
    import concourse.mybir as mybir
    import concourse.tile as tile

    f32 = mybir.dt.float32
    fp16 = mybir.dt.float16
    fp8 = mybir.dt.float8e4
    bf16 = mybir.dt.bfloat16 if use_bf16 else mybir.dt.float16
    EXP = mybir.ActivationFunctionType.Exp

    EC = D // 128           # contraction chunks for the projections
    SC = S // 128           # key/seq chunks
    QB = S // 512           # query blocks of 512
    SH = 1024               # exp tile width (2 psum banks)
    NSH = S // SH           # exp tiles per (sc, h)
    NG = SC // group        # AV accumulation groups
    NPAIR = group * HPC     # (sc, h) pairs per group
    CC = n_devices * K2     # concat width (= D for the real problem)
    NKB = CC // 128         # contraction blocks for WO
    OCB = CC // 128         # output column blocks for WO
    QW = S // n_devices     # per-core query window (512)

    nc = bacc.Bacc("TRN2", target_bir_lowering=False, num_devices=n_devices,
                   enable_partition_id=False)

    xqt = nc.dram_tensor("xqt", [D, S], fp16, kind="ExternalInput")
    xkt = nc.dram_tensor("xkt", [D, S], fp16, kind="ExternalInput")
    xvt = nc.dram_tensor("xvt", [D, S], fp16, kind="ExternalInput")
    wq2 = nc.dram_tensor("wq2", [128, EC * K2], fp16, kind="ExternalInput")
    wk2 = nc.dram_tensor("wk2", [128, EC * K2], fp16, kind="ExternalInput")
    wv2 = nc.dram_tensor("wv2", [128, EC * K2], fp16, kind="ExternalInput")
    qc2 = nc.dram_tensor("qc2", [K2, NCAND], fp16, kind="ExternalInput")
    if tail_mode == "ata":
        wo_c = nc.dram_tensor("wo_c", [128, NKB * OCB * 128], fp16,
                              kind="ExternalInput")
        outT = nc.dram_tensor("outT", [CC, QW], f32, kind="ExternalOutput")
    else:
        wo_c = nc.dram_tensor("wo_c", [CC, 128], fp16, kind="ExternalInput")
        outT = nc.dram_tensor("outT", [128, S], fp16, kind="ExternalOutput")

    with tile.TileContext(nc) as tc:
        with (
            tc.tile_pool(name="const", bufs=1) as const,
            tc.tile_pool(name="main", bufs=1) as main,
            tc.tile_pool(name="xs", bufs=2) as xs,
            tc.tile_pool(name="ep", bufs=3 * group * HPC) as ep,
            tc.tile_pool(name="vp", bufs=2) as vpp,
            tc.tile_pool(name="sm", bufs=20 if act_accum else 8) as sm,
            tc.tile_pool(name="outp", bufs=2) as outp,
            tc.tile_pool(name="dram", bufs=1, space="DRAM") as dram,
        ):
            # ---- weights to SBUF (gpsimd SWDGE queue: keeps SP/DVE/ACT
            # queues free for the x loads and the exp stream) ----
            wq2_sb = const.tile([128, EC, K2], fp16)
            wk2_sb = const.tile([128, EC, K2], fp16)
            wv2_sb = const.tile([128, EC, K2], fp16)
            qc2_sb = const.tile([128, NCAND], fp16)
            if tail_mode == "ata":
                wo_sb = const.tile([128, NKB, OCB, 128], fp16)
            else:
                wo_sb = const.tile([128, NKB, 128], fp16)
            nc.gpsimd.dma_start(wq2_sb[:], wq2.rearrange("p (o k) -> p o k", o=EC))
            nc.gpsimd.dma_start(wk2_sb[:], wk2.rearrange("p (o k) -> p o k", o=EC))
            nc.gpsimd.dma_start(qc2_sb[:], qc2[:, :])
            nc.gpsimd.dma_start(wv2_sb[:], wv2.rearrange("p (o k) -> p o k", o=EC))
            # wo_sb's load is emitted mid-stream (steady state, queue idle)
            # so it doesn't delay the early xtile loads on this queue
            wo_loaded = [False]

            def emit_wo_load():
                if wo_loaded[0]:
                    return
                wo_loaded[0] = True
                if tail_mode == "ata":
                    nc.gpsimd.dma_start(
                        wo_sb[:],
                        wo_c.rearrange("p (a b k) -> p a b k", a=NKB, b=OCB))
                else:
                    nc.gpsimd.dma_start(
                        wo_sb[:], wo_c.rearrange("(o p) k -> p o k", p=128))

            sps = tc.alloc_tile_pool(name="sps", bufs=3, space="PSUM")
            avs = tc.alloc_tile_pool(name="avs", bufs=2, space="PSUM")
            # warm the ACT exp table (its ~2.7us load would otherwise sit
            # on the first real exp's critical path)
            warm = sm.tile([128, 1], f32, tag="rden", name="warm")
            nc.vector.memset(warm[:], 0.0)
            nc.scalar.activation(warm[:], warm[:],
                                 mybir.ActivationFunctionType.Exp)
            for _rep in range(reps):
                # ---- projections (+ per-chunk candidate offsets) ----
                # q2t and heads2 share a 2-buffer tag with rep-alternating
                # allocation order: rep r+1's q2t lands in rep r's heads2
                # buffer (free right after the final flush), so the next
                # rep's Q projection overlaps this rep's AllGather/WO tail.
                def qh_tile(name):
                    return main.tile([128, S], fp16, tag="qh", name=name,
                                     bufs=2)

                if _rep % 2 == 0:
                    q2t = qh_tile("q2t")
                    heads2 = qh_tile("heads2")
                else:
                    heads2 = qh_tile("heads2")
                    q2t = qh_tile("q2t")
                k2t = main.tile([128, S], fp16)
                v2 = main.tile([128, SC, K2], fp16)
                cbias = main.tile([128, SC, HPC], f32)  # -c[s] per (sc, h)
                xq3 = xqt.rearrange("(o p) q -> p o q", p=128)
                xk3 = xkt.rearrange("(o p) q -> p o q", p=128)
                xv3 = xvt.rearrange("(o p) s -> p o s", p=128)
                # fp8 staging target for the final AV accumulation (ag tail)
                cc8 = (main.tile([128, S], fp8, name="cc8")
                       if tail_mode == "ag" and stage8 and upto >= 4 else None)

                def emit_proj_q(qb):
                    # 1024-wide xtiles (2KB contiguous runs per partition —
                    # the DMA-efficiency threshold); DMA on even qb covers
                    # the odd qb too.  q rides the SP HWDGE queue, k rides
                    # the gpsimd queue: ACT's queue carries no transfers.
                    sub = qb % 2
                    if sub == 0:
                        proj_xt["q"] = xs.tile([128, EC, 1024], fp16,
                                               tag="xqk", name="xtile",
                                               bufs=xtile_bufs)
                        nc.sync.dma_start(
                            proj_xt["q"][:],
                            xq3[:, :, (qb // 2) * 1024:(qb // 2 + 1) * 1024])
                    ps = avs.tile([128, 512], f32, tag="av", name="ps_qk")
                    for e in range(EC):
                        nc.tensor.matmul(
                            ps[:], wq2_sb[:, e, :],
                            proj_xt["q"][:, e, sub * 512:(sub + 1) * 512],
                            start=(e == 0), stop=(e == EC - 1),
                        )
                    nc.vector.tensor_copy(q2t[:, qb * 512:(qb + 1) * 512],
                                          ps[:])

                def emit_proj_k(kqb):
                    # K projection + candidate offsets for 512 key cols;
                    # only the first 1024 cols happen in the head — the rest
                    # stream during the attention phase, two groups ahead
                    # of their first use.
                    xkt_t = xs.tile([128, EC, 512], fp16, tag="xk",
                                    name="xktile", bufs=2)
                    nc.gpsimd.dma_start(
                        xkt_t[:], xk3[:, :, kqb * 512:(kqb + 1) * 512])
                    ps = avs.tile([128, 512], f32, tag="av", name="ps_qk")
                    for e in range(EC):
                        nc.tensor.matmul(
                            ps[:], wk2_sb[:, e, :], xkt_t[:, e, :],
                            start=(e == 0), stop=(e == EC - 1),
                        )
                    nc.vector.tensor_copy(k2t[:, kqb * 512:(kqb + 1) * 512],
                                          ps[:])
                    # candidate scores: c[s] = max over candidates
                    for sc in range(kqb * SC // QB, (kqb + 1) * SC // QB) if upto >= 2 else ():
                        for h in range(HPC):
                            ps = avs.tile([128, 512], f32, tag="av",
                                          name="ps_c")
                            nc.tensor.matmul(
                                ps[:, :NCAND],
                                k2t[h * 64:(h + 1) * 64,
                                    sc * 128:(sc + 1) * 128],
                                qc2_sb[h * 64:(h + 1) * 64, :],
                                start=True, stop=True,
                            )
                            nc.vector.tensor_reduce(
                                cbias[:, sc, h:h + 1],
                                ps[:, :NCAND],
                                axis=mybir.AxisListType.X,
                                op=mybir.AluOpType.max, negate=True,
                            )

                def emit_score_exp2(g, scl, t, ets, accs=(None, None)):
                    # h0/h1 matmuls interleaved 1:1 so consecutive PE
                    # instructions target different row groups (64-row
                    # contraction halves) and run concurrently on silicon
                    sc = g * group + scl
                    sp2 = [sps.tile([128, SH], f32, tag="spsum", name="sp")
                           for _ in range(HPC)]
                    for m in range(SH // 512):
                        qo = t * SH + m * 512
                        for h in range(HPC):
                            nc.tensor.matmul(
                                sp2[h][:, m * 512:(m + 1) * 512],
                                k2t[h * 64:(h + 1) * 64,
                                    sc * 128:(sc + 1) * 128],
                                q2t[h * 64:(h + 1) * 64, qo:qo + 512],
                                start=True, stop=True,
                            )
                    for h in range(HPC):
                        nc.scalar.activation(
                            ets[h][:, t * SH:(t + 1) * SH], sp2[h][:], EXP,
                            bias=cbias[:, sc, h:h + 1],
                            accum_out=accs[h],
                        )

                def emit_den(pair, et, den, accq=None):
                    # D'[s] for this pair = sum_q E.
                    if act_accum:
                        nc.vector.tensor_add(
                            den[:, pair:pair + 1], accq[:, 0, :], accq[:, 1, :])
                        for t in range(2, NSH):
                            nc.vector.tensor_add(
                                den[:, pair:pair + 1],
                                den[:, pair:pair + 1], accq[:, t, :])
                    elif ts_accum:
                        nc.vector.tensor_scalar(
                            et[:, :], et[:, :], 1.0, 0.0, mybir.AluOpType.mult,
                            mybir.AluOpType.add,
                            accum_out=den[:, pair:pair + 1],
                        )
                    else:
                        nc.vector.tensor_reduce(
                            den[:, pair:pair + 1], et[:, :],
                            axis=mybir.AxisListType.X, op=mybir.AluOpType.add,
                        )

                def emit_sc_pair(g, scl, den):
                    ets = [ep.tile([128, S], bf16, tag="E", name="et")
                           for _ in range(HPC)]
                    accqs = [None] * HPC
                    if act_accum:
                        accqs = [sm.tile([128, NSH, 1], f32, tag="accq",
                                         name="accq") for _ in range(HPC)]
                    for t in range(NSH):
                        emit_score_exp2(
                            g, scl, t, ets,
                            tuple(a[:, t, :] if act_accum else None
                                  for a in accqs))
                    for h in range(HPC):
                        emit_den(scl * HPC + h, ets[h], den, accqs[h])
                    return ets

                def emit_group_tail(g, e_tiles, den):
                    rden = sm.tile([128, NPAIR], f32, tag="rden", name="rden")
                    nc.vector.reciprocal(rden[:], den[:])
                    vpt = vpp.tile([128, group, K2], bf16, tag="vp",
                                   name="vpt")
                    for scl in range(group):
                        for h in range(HPC):
                            if dve_mul:
                                nc.vector.tensor_scalar_mul(
                                    vpt[:, scl, h * 64:(h + 1) * 64],
                                    v2[:, g * group + scl, h * 64:(h + 1) * 64],
                                    rden[:, scl * HPC + h:scl * HPC + h + 1],
                                )
                            else:
                                nc.scalar.mul(
                                    vpt[:, scl, h * 64:(h + 1) * 64],
                                    v2[:, g * group + scl, h * 64:(h + 1) * 64],
                                    rden[:, scl * HPC + h:scl * HPC + h + 1],
                                )
                    return vpt

                def emit_av_qb(g, e_tiles, vpt, qb):
                    # both heads packed per psum bank via column tiling
                    av = avs.tile([128, 512], f32, tag="av", name="av")
                    if not av_start:
                        nc.vector.memset(av[:], 0.0)
                    for scl in range(group):
                        for h in range(HPC):
                            nc.tensor.matmul(
                                av[h * 64:(h + 1) * 64, :],
                                vpt[:, scl, h * 64:(h + 1) * 64],
                                e_tiles[(scl, h)][:, qb * 512:(qb + 1) * 512],
                                start=(av_start and scl == 0),
                                stop=(scl == group - 1 and h == HPC - 1),
                                skip_group_check=True,
                                tile_position=(0, h * 64),
                            )
                    src = heads2[:, qb * 512:(qb + 1) * 512]
                    if g == NG - 1 and cc8 is not None:
                        # final accumulation renders straight to the fp8
                        # staging tile (halves AllGather + WO read bytes)
                        nc.vector.tensor_add(
                            cc8[:, qb * 512:(qb + 1) * 512], src, av[:])
                    elif g == 0:
                        nc.vector.tensor_copy(src, av[:])
                    else:
                        nc.vector.tensor_add(src, src, av[:])

                def emit_xvg_dma(g):
                    # one group-wide xv DMA (s-chunks, 1KB runs) on the SP
                    # queue, streamed during the attention phase
                    xvg = xs.tile([128, EC, group * 128], fp16, tag="xv",
                                  name="xvg", bufs=1)
                    nc.sync.dma_start(
                        xvg[:], xv3[:, :, g * group * 128:(g + 1) * group * 128])
                    return xvg

                def emit_vproj_chunk(xvg, g, scl):
                    # deferred V projection, one s-chunk at a time so the PE
                    # work spreads across the group instead of bunching at
                    # the boundary (which starved ACT of score tiles)
                    sc = g * group + scl
                    ps = avs.tile([128, 512], f32, tag="av", name="ps_v")
                    for e in range(EC):
                        nc.tensor.matmul(
                            ps[:, :K2],
                            xvg[:, e, scl * 128:(scl + 1) * 128],
                            wv2_sb[:, e, :],
                            start=(e == 0), stop=(e == EC - 1),
                        )
                    nc.vector.tensor_copy(v2[:, sc, :], ps[:, :K2])

                # ---- AV work queue: groups whose den/vpt are ready get
                # their 8 AV q-blocks drained a few at a time per pair, so
                # PE work spreads evenly and E tiles retire steadily ----
                pending = []  # [g, tiles, vpt, next_qb]

                def drain_av(n):
                    done = 0
                    while pending and done < n:
                        ent = pending[0]
                        emit_av_qb(ent[0], ent[1], ent[2], qb=ent[3])
                        ent[3] += 1
                        done += 1
                        if ent[3] == QB:
                            pending.pop(0)

                # ---- projections, interleaved with the first head_groups
                # groups' attention.  Their exp t-rounds are emitted as soon
                # as the q2t columns they read are projected, so the ACT exp
                # stream saturates while the (DMA-bound) projection phase is
                # still streaming inputs.
                HG = min(head_groups, NG) if upto >= 3 else 0
                # k columns needed by steady group g are [g*group*128,
                # (g+1)*group*128); head covers kqb 0..HKQB-1, the rest are
                # emitted two groups ahead of first use (group==2 only —
                # other group sizes project everything in the head).
                if upto >= 3 and group == 2:
                    HKQB = 2
                else:
                    HKQB = QB
                ksched = {}  # steady group -> kqb to project during it
                for kqb in range(HKQB, QB):
                    ksched[(kqb * 512) // (group * 128) - 2] = kqb
                proj_xt = {}
                hg_tiles = {}  # (g, scl, h) -> E tile
                hg_accq = {}
                hg_dens = {}
                xvgs = {}
                for qb in range(QB):
                    emit_proj_q(qb)
                    if qb < HKQB:
                        emit_proj_k(qb)
                    if upto < 3:
                        continue
                    if qb == 0:
                        for g in range(HG):
                            xvgs[g] = emit_xvg_dma(g)
                            hg_dens[g] = sm.tile([128, NPAIR], f32,
                                                 tag="den", name="den")
                    if qb % 2 == 1:
                        t = qb // 2
                        for g in range(HG):
                            for scl in range(group):
                                for h in range(HPC):
                                    if (g, scl, h) not in hg_tiles:
                                        hg_tiles[(g, scl, h)] = ep.tile(
                                            [128, S], bf16, tag="E",
                                            name="et")
                                        if act_accum:
                                            hg_accq[(g, scl, h)] = sm.tile(
                                                [128, NSH, 1], f32,
                                                tag="accq", name="accq")
                                emit_score_exp2(
                                    g, scl, t,
                                    [hg_tiles[(g, scl, h)]
                                     for h in range(HPC)],
                                    tuple(hg_accq[(g, scl, h)][:, t, :]
                                          if act_accum else None
                                          for h in range(HPC)))
                        i = qb // 2
                        if i < HG * group:
                            emit_vproj_chunk(xvgs[i // group], i // group,
                                             i % group)
                if upto < 3:
                    continue
                for g in range(HG):
                    for scl in range(group):
                        for h in range(HPC):
                            emit_den(scl * HPC + h, hg_tiles[(g, scl, h)],
                                     hg_dens[g], hg_accq.get((g, scl, h)))
                    tiles_g = {(scl, h): hg_tiles[(g, scl, h)]
                               for scl in range(group) for h in range(HPC)}
                    vpt_g = emit_group_tail(g, tiles_g, hg_dens[g])
                    pending.append([g, tiles_g, vpt_g, 0])

                for g in range(HG, NG):
                    e_tiles = {}
                    xvg = emit_xvg_dma(g)
                    if g in ksched:
                        emit_proj_k(ksched[g])
                    if g == HG:
                        emit_wo_load()
                    den = sm.tile([128, NPAIR], f32, tag="den", name="den")
                    for scl in range(group):
                        ets = emit_sc_pair(g, scl, den)
                        for h in range(HPC):
                            e_tiles[(scl, h)] = ets[h]
                        emit_vproj_chunk(xvg, g, scl)
                        drain_av(drain_max)
                    vpt = emit_group_tail(g, e_tiles, den)
                    pending.append([g, e_tiles, vpt, 0])

                # ---- final AV flush + cross-core redistribution ----
                if tail_mode == "ata":
                    # heads2[:, qw_d] goes to core d; one small (1MB)
                    # AllToAll; each core then computes the full 1024-wide
                    # WO for its own 512-query window.
                    ata_in = dram.tile([CC, QW], fp16, name="ata_in")
                    ata_out = dram.tile([CC, QW], fp16, name="ata_out")
                    NQW = QW // 512

                    def stage_qb(qb):
                        d, lo = qb // NQW, (qb % NQW) * 512
                        nc.sync.dma_start(
                            ata_in[d * 128:(d + 1) * 128, lo:lo + 512],
                            heads2[:, qb * 512:(qb + 1) * 512])
                else:
                    # v2-style: stage heads2 to DRAM, AllGather the full
                    # concat in ag_split row parts (WO matmuls over part 0
                    # overlap the later gathers; wo_c rows pre-permuted on
                    # the host to match), column-sharded WO.
                    cc_dt = fp8 if stage8 else fp16
                    cc_src = cc8 if stage8 else heads2
                    cc_in = dram.tile([128, S], cc_dt, name="cc_in")
                    space = "Local" if (fake_ag or n_devices <= 4) else "Shared"
                    RPP = 128 // ag_split
                    cc_part = [dram.tile([CC // ag_split, S], cc_dt,
                                         addr_space=space, name=f"cc_part{i}")
                               for i in range(ag_split)]

                    def stage_qb(qb):
                        nc.sync.dma_start(
                            cc_in[:, qb * 512:(qb + 1) * 512],
                            cc_src[:, qb * 512:(qb + 1) * 512])

                while pending:
                    last = len(pending) == 1
                    qb = pending[0][3]
                    drain_av(1)
                    if last and upto >= 4:
                        stage_qb(qb)
                if upto < 4:
                    continue

                if tail_mode == "ata":
                    if fake_ag:
                        nc.sync.dma_start(ata_out[:, :], ata_in[:, :])
                    else:
                        nc.gpsimd.collective_compute(
                            "AllToAll", mybir.AluOpType.bypass,
                            replica_groups=[list(range(n_devices))],
                            ins=[ata_in[:, :].opt()],
                            outs=[ata_out[:, :].opt()],
                        )
                else:
                    for i in range(ag_split):
                        part = cc_in[i * RPP:(i + 1) * RPP, :]
                        if fake_ag:
                            nc.sync.dma_start(cc_part[i][:RPP, :], part)
                        else:
                            nc.gpsimd.collective_compute(
                                "AllGather", mybir.AluOpType.bypass,
                                replica_groups=[list(range(n_devices))],
                                ins=[part.opt()], outs=[cc_part[i].opt()],
                            )

                if upto < 5:
                    continue
                if tail_mode == "ata":
                    # ---- WO: full 1024 columns for this core's 512-query
                    # window, contraction over the gathered concat.  att_sb
                    # borrows an E-pool buffer (E tiles are dead here) ----
                    att_sb = ep.tile([128, NKB, QW], fp16, tag="E", name="att")
                    ata3 = ata_out.rearrange("(o p) q -> p o q", p=128)
                    for kb in range(NKB):
                        nc.sync.dma_start(att_sb[:, kb, :], ata3[:, kb, :])
                    for j in range(OCB // 2):
                        pq = sps.tile([128, 1024], f32, tag="spsum", name="pq")
                        for kb in range(NKB):
                            for i in (0, 1):
                                nc.tensor.matmul(
                                    pq[:, i * 512:(i + 1) * 512],
                                    wo_sb[:, kb, 2 * j + i, :],
                                    att_sb[:, kb, :],
                                    start=(kb == 0), stop=(kb == NKB - 1),
                                )
                        for i in (0, 1):
                            ocb = 2 * j + i
                            osb = outp.tile([128, 512], f32, tag="osb",
                                            name="osb")
                            nc.vector.tensor_copy(
                                osb[:], pq[:, i * 512:(i + 1) * 512])
                            nc.sync.dma_start(
                                outT[ocb * 128:(ocb + 1) * 128, :], osb[:])
                else:
                    # ---- WO: this core's 128 output columns over all S
                    # queries; 2048-wide ccr chunks on the (now idle) ACT
                    # HWDGE queue; 4 psum chains in two 1024-wide tiles ----
                    QWW = 4  # 512-blocks per ccr chunk
                    WCC = QWW * 512
                    cpp = NKB // ag_split  # 128-row chunks per AG part
                    for qbo in range(QB // QWW):
                        pqs = [sps.tile([128, 1024], f32, tag="spsum",
                                        name="pq") for _ in range(QWW // 2)]
                        for kb in range(NKB):
                            ccr = xs.tile([128, WCC], cc_dt, tag="ccr",
                                          name="ccr", bufs=2)
                            qw = slice(qbo * WCC, (qbo + 1) * WCC)
                            eng = nc.scalar if kb % 2 == 0 else nc.gpsimd
                            eng.dma_start(
                                ccr[:], cc_part[kb // cpp][
                                    (kb % cpp) * 128:(kb % cpp) * 128 + 128,
                                    qw])
                            for qi in range(QWW):
                                nc.tensor.matmul(
                                    pqs[qi // 2][:, (qi % 2) * 512:(qi % 2 + 1) * 512],
                                    wo_sb[:, kb, :],
                                    ccr[:, qi * 512:(qi + 1) * 512],
                                    start=(kb == 0), stop=(kb == NKB - 1),
                                )
                        for qi in range(QWW):
                            qb = qbo * QWW + qi
                            osb = outp.tile([128, 512], fp16, tag="osb",
                                            name="osb")
                            nc.vector.tensor_copy(
                                osb[:], pqs[qi // 2][:, (qi % 2) * 512:(qi % 2 + 1) * 512])
                            nc.sync.dma_start(
                                outT[:, qb * 512:(qb + 1) * 512], osb[:])
            avs.release()
            sps.release()

    nc.compile()
    return nc


def make_core_inputs(XQ, XK, XV, WQ_comb, WK_comb, WV_comb, WQh, WKh, WVh, WO,
                     n_cores=N_CORES, hpc=HPC, tail_mode=None, ag_split=1):
    """Host-side shard/layout prep. Returns in_maps for run_bass_kernel_spmd."""
    tail_mode = tail_mode or TAIL_MODE
    f32 = np.float32
    xqt = np.ascontiguousarray(np.asarray(XQ, f32).T).astype(FP16)
    xkt = np.ascontiguousarray(np.asarray(XK, f32).T).astype(FP16)
    xvt = np.ascontiguousarray(np.asarray(XV, f32).T).astype(FP16)
    XQ = np.asarray(XQ, f32)
    xkbar = np.asarray(XK, f32).mean(axis=0)
    WQ_comb = np.asarray(WQ_comb, f32)
    WK_comb = np.asarray(WK_comb, f32)
    WV_comb = np.asarray(WV_comb, f32)
    WQh, WKh, WVh = np.asarray(WQh, f32), np.asarray(WKh, f32), np.asarray(WVh, f32)
    WO = np.asarray(WO, f32)
    strided = np.arange(0, XQ.shape[0], 128)
    EC = D_MODEL // 128
    NKB = (n_cores * K2) // 128

    def pmajor(w):  # [D, K2] -> [128, EC*K2] with p-major contiguous layout
        return np.ascontiguousarray(
            w.reshape(EC, 128, K2).transpose(1, 0, 2).reshape(128, EC * K2))

    if tail_mode == "ata":
        # full WO, p-major [p, kb, ocb, col], replicated to every core
        wo_full = np.ascontiguousarray(
            WO.reshape(NKB, 128, NKB, 128).transpose(1, 0, 2, 3).reshape(128, -1)
        ).astype(FP16)
    else:
        # WO row permutation matching the N-way split AllGather layout:
        # [all cores' local rows of part 0; ...; part N-1]
        rpp = 128 // ag_split
        wo_perm = np.concatenate(
            [np.arange(c * 128 + p * rpp, c * 128 + (p + 1) * rpp)
             for p in range(ag_split) for c in range(n_cores)])

    in_maps = []
    for c in range(n_cores):
        hs = slice(c * hpc, (c + 1) * hpc)
        # stack this core's heads along columns, then fold the combined proj;
        # the softmax 1/sqrt(dk) goes into the Q weights
        wq2 = (WQ_comb @ np.concatenate(list(WQh[hs]), axis=1)) / np.sqrt(D_K)
        wk2 = WK_comb @ np.concatenate(list(WKh[hs]), axis=1)
        wv2 = WV_comb @ np.concatenate(list(WVh[hs]), axis=1)
        k2 = wq2.shape[1]
        wq2_16, wk2_16 = wq2.astype(FP16), wk2.astype(FP16)
        # candidate queries per head: both tails of the cheap proxy
        # a[q] = XQ @ (wq2_h @ (xkbar @ wk2_h)) plus a strided sample;
        # project them exactly like the device does (fp16 operands,
        # f32 accumulate, fp16 result)
        qc2 = np.empty((k2, NCAND), FP16)
        for h in range(hpc):
            cs = slice(h * D_K, (h + 1) * D_K)
            u = xkbar @ wk2[:, cs]
            a = XQ @ (wq2[:, cs] @ u)
            o = np.argsort(a)
            cand = np.concatenate([o[:16], o[-16:], strided])[:NCAND]
            qc = (XQ[cand].astype(FP16).astype(f32)
                  @ wq2_16[:, cs].astype(f32)).astype(FP16)  # [NCAND, 64]
            qc2[cs, :] = qc.T
        if tail_mode == "ata":
            wo_c = wo_full
        else:
            wo_c = np.ascontiguousarray(
                WO[wo_perm][:, c * k2:(c + 1) * k2]).astype(FP16)
        in_maps.append({
            "xqt": xqt, "xkt": xkt, "xvt": xvt,
            "wq2": pmajor(wq2).astype(FP16),
            "wk2": pmajor(wk2).astype(FP16),
            "wv2": pmajor(wv2).astype(FP16),
            "qc2": qc2,
            "wo_c": wo_c,
        })
    return in_maps


_PROGRAM_CACHE = {}


def _get_program(D, S, n_devices):
    key = (D, S, n_devices)
    if key not in _PROGRAM_CACHE:
        _PROGRAM_CACHE[key] = build_program(D, S, n_devices)
    return _PROGRAM_CACHE[key]


def kernel(XQ, XK, XV, WQ_comb, WK_comb, WV_comb, WQh, WKh, WVh, WO,
           _trace=False):
    from concourse.bass_utils import run_bass_kernel_spmd

    in_maps = make_core_inputs(XQ, XK, XV, WQ_comb, WK_comb, WV_comb,
                               WQh, WKh, WVh, WO)
    nc = _get_program(D_MODEL, SEQ, N_CORES)
    res = run_bass_kernel_spmd(nc, in_maps, core_ids=list(range(N_CORES)),
                               trace=_trace)
    out = np.empty((SEQ, D_MODEL), np.float32)
    if TAIL_MODE == "ata":
        QW = SEQ // N_CORES
        for c in range(N_CORES):
            out[c * QW:(c + 1) * QW, :] = res.results[c]["outT"].T
    else:
        for c in range(N_CORES):
            out[:, c * 128:(c + 1) * 128] = res.results[c]["outT"].T
    if _trace:
        kernel.last_results = res
    return out
